# revision 12
# baseline (speedup 1.0000x reference)
"""Trainium2 Bass kernel for BiBo attention (GQA + per-head RMSNorm + RoPE +
SSMax scaling + causal attention + o_proj).

Sharding: tensor-parallel over the 4 KV-head groups x data-parallel over the
2 batch elements = 8 cores. Each core computes its 4 q-heads / 1 kv-head of
attention for one batch element plus its row-slice of o_proj; the host sums
the 4 partial o_proj outputs per batch element (row-parallel unshard).

Layout strategy (per core):
  - hidden^T [H, S] streamed from DRAM; projections produce q^T/k^T with the
    head dim on partitions so QK^T needs no transposes.
  - scores are computed transposed (scoresT[k, q]) so the PV matmul consumes
    exp(scoresT) directly; the softmax denominator is a ones-vector matmul
    (partition-dim sum on the PE), and no max-subtraction is needed because
    RMS-normed q/k bound |scores| <= sqrt(HD)*ssmax*log(S) ~ 10.
  - block-sparse causal skipping: mask blocks that are entirely <= -1e8 are
    skipped (their exp underflows to exactly 0 in fp32); all-zero blocks skip
    the mask add; others add the real mask values. The block plan is derived
    from the actual attention_mask at call time.
"""

import math

import numpy as np

B, S, H = 2, 2048, 2048
NH, NKV, HD = 16, 4, 128
EPS = 1e-6
NCORES = 8
TP = 4            # kv-head groups
QH = NH // NKV    # q heads per core
SC = 512          # q-tile / s-chunk width
NSC = S // SC     # 4
KT = 128          # k tile
NKT = S // KT     # 16
HC = 128          # h contraction chunk
NHC = H // HC     # 16
SKIP_THRESH = -1e8

_compiled_cache = {}
LAST_EXEC_NS = None
LAST_RESULTS = None


def _build_program(plan, mask_counts):
    import concourse.mybir as mybir
    import concourse.tile as tile
    from concourse import bacc

    F32 = mybir.dt.float32
    F32R = mybir.dt.float32r
    MM = mybir.dt.bfloat16
    AF = mybir.ActivationFunctionType
    OP = mybir.AluOpType

    n_mask = sum(mask_counts)

    nc = bacc.Bacc("TRN2", target_bir_lowering=False, debug=False,
                   num_devices=NCORES)
    hT = nc.dram_tensor("hT", [H, S], MM, kind="ExternalInput").ap()
    wqT = nc.dram_tensor("wqT", [H, QH * HD], MM, kind="ExternalInput").ap()
    wkT = nc.dram_tensor("wkT", [H, HD], MM, kind="ExternalInput").ap()
    wvT = nc.dram_tensor("wvT", [H, HD], MM, kind="ExternalInput").ap()
    woT = nc.dram_tensor("woT", [QH * HD, H], MM, kind="ExternalInput").ap()
    cosT = nc.dram_tensor("cosT", [HD, S], F32, kind="ExternalInput").ap()
    sinT = nc.dram_tensor("sinT", [HD, S], F32, kind="ExternalInput").ap()
    sgn = nc.dram_tensor("sgn", [HD, 1], F32, kind="ExternalInput").ap()
    qc = nc.dram_tensor("qc", [1, QH * SC], F32, kind="ExternalInput").ap()
    iwq = nc.dram_tensor("iwq", [HD, 1], MM, kind="ExternalInput").ap()
    iwk = nc.dram_tensor("iwk", [HD, 1], MM, kind="ExternalInput").ap()
    if n_mask:
        mblk = nc.dram_tensor("mblk", [n_mask, KT, SC], F32,
                              kind="ExternalInput").ap()
    out = nc.dram_tensor("out", [S, H], F32, kind="ExternalOutput").ap()

    with tile.TileContext(nc) as tc:
        _emit(nc, tc, locals(), plan, mask_counts, MM, F32, F32R, AF, OP)
    nc.compile()
    return nc


def _emit(nc, tc, T, plan, mask_counts, MM, F32, F32R, AF, OP):
    from contextlib import ExitStack

    hT, wqT, wkT, wvT, woT = T["hT"], T["wqT"], T["wkT"], T["wvT"], T["woT"]
    cosT, sinT, sgn, qc = T["cosT"], T["sinT"], T["sgn"], T["qc"]
    iwq, iwk, out = T["iwq"], T["iwk"], T["out"]
    mblk = T.get("mblk")

    ctx = ExitStack()
    with ctx:
        const = ctx.enter_context(tc.tile_pool(name="const", bufs=1))
        wpool = ctx.enter_context(tc.tile_pool(name="w", bufs=1))
        persist = ctx.enter_context(tc.tile_pool(name="persist", bufs=1))
        hpool = ctx.enter_context(tc.tile_pool(name="h", bufs=18))
        mpool = ctx.enter_context(tc.tile_pool(name="m", bufs=6))
        spool = ctx.enter_context(tc.tile_pool(name="s", bufs=2))
        epool = ctx.enter_context(tc.tile_pool(name="e", bufs=3))
        atpool = ctx.enter_context(tc.tile_pool(name="at", bufs=8))
        opool_sb = ctx.enter_context(tc.tile_pool(name="osb", bufs=3))
        ps_mm = ctx.enter_context(tc.tile_pool(name="psmm", bufs=2, space="PSUM"))
        ps_v = ctx.enter_context(tc.tile_pool(name="psv", bufs=1, space="PSUM"))
        ps_pv = ctx.enter_context(tc.tile_pool(name="pspv", bufs=2, space="PSUM"))
        ps_es = ctx.enter_context(tc.tile_pool(name="pses", bufs=1, space="PSUM"))
        ps_o = ctx.enter_context(tc.tile_pool(name="pso", bufs=2, space="PSUM"))

        # ---- persistent tiles (loads emitted by the driver below) -------
        wq_t = wpool.tile([128, NHC * QH * HD], MM, tag="wq")
        wk_t = wpool.tile([128, NHC * HD], MM, tag="wk")
        wv_t = wpool.tile([128, NHC * HD], MM, tag="wv")
        wo_t = wpool.tile([128, QH * H], MM, tag="wo")
        cos_t = wpool.tile([128, S], F32, tag="cos")
        sin_t = wpool.tile([128, S], F32, tag="sin")
        sgn_t = const.tile([128, 1], F32, tag="sgn")
        qc_t = const.tile([1, QH * SC], F32, tag="qc")
        iwq_t = const.tile([128, 1], MM, tag="iwq")
        iwk_t = const.tile([128, 1], MM, tag="iwk")
        ones_t = const.tile([128, 1], MM, tag="ones")
        eps_t = const.tile([1, 1], F32, tag="eps")
        khat = persist.tile([128, S], MM, tag="khat")
        v_sb = persist.tile([128, S], MM, tag="v")
        qhat = [persist.tile([128, S], MM, name=f"qhat{i}", tag=f"qhat{i}")
                for i in range(QH)]

        def load_early():
            for c in range(NHC):
                nc.sync.dma_start(wq_t[:, c * QH * HD:(c + 1) * QH * HD],
                                  wqT[c * HC:(c + 1) * HC, :])
                nc.sync.dma_start(wk_t[:, c * HD:(c + 1) * HD],
                                  wkT[c * HC:(c + 1) * HC, :])
                nc.sync.dma_start(wv_t[:, c * HD:(c + 1) * HD],
                                  wvT[c * HC:(c + 1) * HC, :])
            nc.sync.dma_start(sgn_t[:], sgn[:])
            nc.sync.dma_start(qc_t[:], qc[:])
            nc.sync.dma_start(iwq_t[:], iwq[:])
            nc.sync.dma_start(iwk_t[:], iwk[:])
            nc.vector.memset(ones_t[:], 1.0)
            nc.vector.memset(eps_t[:], EPS)
            nc.sync.dma_start(cos_t[:], cosT[:])
            nc.sync.dma_start(sin_t[:], sinT[:])

        def load_wo():
            for f in range(QH):
                nc.sync.dma_start(wo_t[:, f * H:(f + 1) * H],
                                  woT[f * HD:(f + 1) * HD, :])

        # per-(head,chunk) norm+rope: PSUM proj tile -> SBUF hat tile
        def norm_rope(pp, sc, iw_t, qconst_slice, hat_dst):
            sh = spool.tile([128, SC], F32, tag="sh")
            nc.vector.tensor_copy(sh[0:64, :], pp[64:128, :])
            nc.vector.tensor_copy(sh[64:128, :], pp[0:64, :])
            # var from the rotated copy (rotation-invariant; iw_t rows are
            # pre-rotated host-side to match)
            sq = spool.tile([128, SC], MM, tag="sq")
            nc.vector.tensor_mul(sq[:], sh[:], sh[:])
            var = ps_mm.tile([1, SC], F32, tag="mm")
            nc.tensor.matmul(var[:], iw_t[:], sq[:], start=True, stop=True)
            sd = spool.tile([1, SC], F32, tag="sd")
            nc.scalar.activation(sd[:], var[:], AF.Sqrt, bias=eps_t[:])
            rs = spool.tile([1, SC], F32, tag="rs")
            nc.vector.reciprocal_approx_fast(rs[:], sd[:])
            if qconst_slice is not None:
                nc.vector.tensor_mul(rs[:], rs[:], qconst_slice)
            bb = spool.tile([128, SC], F32, tag="bb")
            nc.gpsimd.partition_broadcast(bb[:], rs[:], 128)
            tt = spool.tile([128, SC], F32, tag="tt")
            nc.vector.scalar_tensor_tensor(
                tt[:], sh[:], sgn_t[:], sin_t[:, sc * SC:(sc + 1) * SC],
                op0=OP.mult, op1=OP.mult)
            uu = spool.tile([128, SC], F32, tag="uu")
            nc.vector.tensor_mul(uu[:], pp[:], cos_t[:, sc * SC:(sc + 1) * SC])
            nc.vector.tensor_add(tt[:], tt[:], uu[:])
            nc.vector.tensor_mul(hat_dst, tt[:], bb[:])

        # ---- projections, per s-chunk -----------------------------------
        def hts_load(sc):
            hts = []
            for c in range(NHC):
                t = hpool.tile([128, SC], MM, tag="ht", name="ht")
                nc.sync.dma_start(t[:], hT[c * HC:(c + 1) * HC,
                                           sc * SC:(sc + 1) * SC])
                hts.append(t)
            return hts

        def proj_chunk(sc, hts):
            # v-proj: natural [s, d] layout, N=128 matmuls
            for ss in range(4):
                vp = ps_v.tile([128, HD], F32, tag="v")
                for c in range(NHC):
                    nc.tensor.matmul(vp[:], hts[c][:, ss * 128:(ss + 1) * 128],
                                     wv_t[:, c * HD:(c + 1) * HD],
                                     start=(c == 0), stop=(c == NHC - 1))
                col = (sc * 4 + ss) * 128
                nc.vector.tensor_copy(v_sb[:, col:col + 128], vp[:])
            # k-proj
            kp = ps_mm.tile([128, SC], F32, tag="mm")
            for c in range(NHC):
                nc.tensor.matmul(kp[:], wk_t[:, c * HD:(c + 1) * HD],
                                 hts[c][:], start=(c == 0), stop=(c == NHC - 1))
            norm_rope(kp, sc, iwk_t, None, khat[:, sc * SC:(sc + 1) * SC])
            # q-proj per head
            for hd in range(QH):
                qp = ps_mm.tile([128, SC], F32, tag="mm")
                for c in range(NHC):
                    base = c * QH * HD + hd * HD
                    nc.tensor.matmul(qp[:], wq_t[:, base:base + HD], hts[c][:],
                                     start=(c == 0), stop=(c == NHC - 1))
                norm_rope(qp, sc, iwq_t,
                          qc_t[:, hd * SC:(hd + 1) * SC],
                          qhat[hd][:, sc * SC:(sc + 1) * SC])

        # ---- attention + o_proj, per q-tile ------------------------------
        mask_starts = [sum(mask_counts[:i]) for i in range(NSC)]

        def attn_qtile(qi):
            mask_idx = mask_starts[qi]
            kts = [kt for kt in range(NKT) if plan[qi][kt] != "skip"]
            # load this q-tile's mask blocks (shared across heads)
            mtiles = {}
            for kt in kts:
                if plan[qi][kt] == "mask":
                    mt = mpool.tile([128, SC], F32, tag="mask")
                    nc.sync.dma_start(mt[:], mblk[mask_idx])
                    mtiles[kt] = mt
                    mask_idx += 1
            ats = []
            for hd in range(QH):
                qsl = qhat[hd][:, qi * SC:(qi + 1) * SC]
                pv = ps_pv.tile([128, SC], F32, tag="pv")
                es = ps_es.tile([1, SC], F32, tag="es")
                sts = {}
                # pipeline QK^T one k-tile ahead of exp/PV
                for j, kt in enumerate(kts):
                    st = ps_mm.tile([128, SC], F32, tag="mm")
                    nc.tensor.matmul(st[:], khat[:, kt * 128:(kt + 1) * 128],
                                     qsl, start=True, stop=True)
                    if kt in mtiles:
                        nc.vector.tensor_add(st[:], st[:], mtiles[kt][:])
                    sts[j] = st
                    if j >= 1:
                        _attn_tail(nc, j - 1, kts, sts, es, pv, v_sb,
                                   ones_t, MM, AF, epool)
                _attn_tail(nc, len(kts) - 1, kts, sts, es, pv, v_sb,
                           ones_t, MM, AF, epool)
                rs = spool.tile([1, SC], F32, tag="ars")
                nc.vector.reciprocal_approx_fast(rs[:], es[:])
                bb = spool.tile([128, SC], F32, tag="abb")
                nc.gpsimd.partition_broadcast(bb[:], rs[:], 128)
                at = atpool.tile([128, SC], MM, tag="at")
                nc.vector.tensor_mul(at[:], pv[:], bb[:])
                ats.append(at)
            # o_proj for this q-tile
            for ss in range(4):
                for ho in range(4):
                    op_t = ps_o.tile([128, SC], F32, tag="o")
                    for hd in range(QH):
                        nc.tensor.matmul(
                            op_t[:],
                            ats[hd][:, ss * 128:(ss + 1) * 128],
                            wo_t[:, hd * H + ho * SC:hd * H + (ho + 1) * SC],
                            start=(hd == 0), stop=(hd == QH - 1))
                    ob = opool_sb.tile([128, SC], F32, tag="osb")
                    nc.vector.tensor_copy(ob[:], op_t[:])
                    nc.sync.dma_start(
                        out[qi * SC + ss * 128:qi * SC + (ss + 1) * 128,
                            ho * SC:(ho + 1) * SC],
                        ob[:])

        # ---- driver: software-pipelined phase order ----------------------
        hts0 = hts_load(0)
        load_early()
        proj_chunk(0, hts0)
        hts1 = hts_load(1)
        proj_chunk(1, hts1)
        load_wo()
        attn_qtile(0)
        hts2 = hts_load(2)
        proj_chunk(2, hts2)
        attn_qtile(1)
        hts3 = hts_load(3)
        proj_chunk(3, hts3)
        attn_qtile(2)
        attn_qtile(3)


def _attn_tail(nc, j, kts, sts, es, pv, v_sb, ones_t, MM, AF, epool):
    """exp + PV + PSUM-accumulated denominator for pipelined k-tile j."""
    kt = kts[j]
    st = sts.pop(j)
    ex = epool.tile([128, SC], MM, tag="ex", name="ex")
    nc.scalar.activation(ex[:], st[:], AF.Exp)
    last = j == len(kts) - 1
    nc.tensor.matmul(pv[:], v_sb[:, kt * 128:(kt + 1) * 128], ex[:],
                     start=(j == 0), stop=last)
    nc.tensor.matmul(es[:], ones_t[:], ex[:], start=(j == 0), stop=last)


def _mask_plan(mask):
    """Classify [qi][kt] blocks of the (q,k) mask, unified across batch."""
    plan = []
    for qi in range(NSC):
        row = []
        for kt in range(NKT):
            blk = mask[:, 0, qi * SC:(qi + 1) * SC, kt * KT:(kt + 1) * KT]
            if (blk <= SKIP_THRESH).all():
                row.append("skip")
            elif (blk == 0.0).all():
                row.append("zero")
            else:
                row.append("mask")
        # guard: a q-tile with no included block would divide by zero
        if all(s == "skip" for s in row):
            row[0] = "mask"
        plan.append(row)
    return plan


def kernel(hidden_states, cos, sin, attention_mask, wq, wk, wv, wo,
           q_norm_w, k_norm_w, ssmax_scale):
    global LAST_EXEC_NS
    import os
    import ml_dtypes
    from concourse.bass_utils import run_bass_kernel_spmd

    f32 = np.float32
    hidden_states = np.asarray(hidden_states, f32)
    cos = np.asarray(cos, f32)
    sin = np.asarray(sin, f32)
    attention_mask = np.asarray(attention_mask, f32)
    wq = np.asarray(wq, f32)
    wk = np.asarray(wk, f32)
    wv = np.asarray(wv, f32)
    wo = np.asarray(wo, f32)
    q_norm_w = np.asarray(q_norm_w, f32)
    k_norm_w = np.asarray(k_norm_w, f32)
    ssmax = np.asarray(ssmax_scale, f32).reshape(NH)

    plan = _mask_plan(attention_mask)
    mask_counts = [sum(1 for s in row if s == "mask") for row in plan]
    key = (tuple(tuple(r) for r in plan),)
    if key not in _compiled_cache:
        _compiled_cache[key] = _build_program(plan, mask_counts)
    nc = _compiled_cache[key]

    bf16 = ml_dtypes.bfloat16
    qw = np.tile(q_norm_w, QH)
    sgn_np = np.concatenate([-np.ones(64, f32), np.ones(64, f32)])[:, None]
    iwq_np = np.roll(1.0 / (HD * q_norm_w ** 2), -64).astype(bf16)[:, None]
    iwk_np = np.roll(1.0 / (HD * k_norm_w ** 2), -64).astype(bf16)[:, None]
    cosT_np = np.ascontiguousarray(cos.T)
    sinT_np = np.ascontiguousarray(sin.T)

    in_maps = []
    for core in range(NCORES):
        b, g = divmod(core, TP)
        hTm = np.ascontiguousarray(hidden_states[b].T).astype(bf16)
        wq_s = wq[g * QH * HD:(g + 1) * QH * HD] * qw[:, None]
        wk_s = wk[g * HD:(g + 1) * HD] * k_norm_w[:, None]
        wv_s = wv[g * HD:(g + 1) * HD]
        wo_s = wo[:, g * QH * HD:(g + 1) * QH * HD]
        qc_np = np.empty((1, QH * SC), f32)
        for i in range(QH):
            qc_np[0, i * SC:(i + 1) * SC] = (
                ssmax[g * QH + i] * math.log(S) / math.sqrt(HD))
        m = {
            "hT": hTm,
            "wqT": np.ascontiguousarray(wq_s.T).astype(bf16),
            "wkT": np.ascontiguousarray(wk_s.T).astype(bf16),
            "wvT": np.ascontiguousarray(wv_s.T).astype(bf16),
            "woT": np.ascontiguousarray(wo_s.T).astype(bf16),
            "cosT": cosT_np, "sinT": sinT_np, "sgn": sgn_np,
            "qc": qc_np, "iwq": iwq_np, "iwk": iwk_np,
        }
        n_mask = sum(mask_counts)
        if n_mask:
            blocks = np.empty((n_mask, KT, SC), f32)
            i = 0
            for qi in range(NSC):
                for kt in range(NKT):
                    if plan[qi][kt] == "mask":
                        blocks[i] = attention_mask[
                            b, 0, qi * SC:(qi + 1) * SC,
                            kt * KT:(kt + 1) * KT].T
                        i += 1
            m["mblk"] = blocks
        in_maps.append(m)

    trace = bool(int(os.environ.get("BASS_KERNEL_TRACE", "0")))
    res = run_bass_kernel_spmd(nc, in_maps, list(range(NCORES)), trace=trace)
    LAST_EXEC_NS = res.exec_time_ns
    globals()["LAST_RESULTS"] = res

    final = np.zeros((B, S, H), f32)
    for core in range(NCORES):
        b = core // TP
        final[b] += res.results[core]["out"]
    return final


# revision 18
# speedup vs baseline: 1.0470x; 1.0470x over previous
"""Trainium2 Bass kernel for BiBo attention (GQA + per-head RMSNorm + RoPE +
SSMax scaling + causal attention + o_proj).

Sharding: tensor-parallel over the 4 KV-head groups x data-parallel over the
2 batch elements = 8 cores. Each core computes its 4 q-heads / 1 kv-head of
attention for one batch element plus its row-slice of o_proj; the host sums
the 4 partial o_proj outputs per batch element (row-parallel unshard).

Layout strategy (per core):
  - hidden^T [H, S] streamed from DRAM; projections produce q^T/k^T with the
    head dim on partitions so QK^T needs no transposes.
  - scores are computed transposed (scoresT[k, q]) so the PV matmul consumes
    exp(scoresT) directly; the softmax denominator is a ones-vector matmul
    (partition-dim sum on the PE), and no max-subtraction is needed because
    RMS-normed q/k bound |scores| <= sqrt(HD)*ssmax*log(S) ~ 10.
  - block-sparse causal skipping: mask blocks that are entirely <= -1e8 are
    skipped (their exp underflows to exactly 0 in fp32); all-zero blocks skip
    the mask add; others add the real mask values. The block plan is derived
    from the actual attention_mask at call time.
"""

import math

import numpy as np

B, S, H = 2, 2048, 2048
NH, NKV, HD = 16, 4, 128
EPS = 1e-6
NCORES = 8
TP = 4            # kv-head groups
QH = NH // NKV    # q heads per core
SC = 512          # q-tile / s-chunk width
NSC = S // SC     # 4
KT = 128          # k tile
NKT = S // KT     # 16
HC = 128          # h contraction chunk
NHC = H // HC     # 16
SKIP_THRESH = -1e8

_compiled_cache = {}
LAST_EXEC_NS = None
LAST_RESULTS = None


def _enable_ldw_opt():
    import os
    if not os.environ.get("BASS_LDW_OPT"):
        return
    from concourse import bass_utils as bu
    if getattr(bu.run_command, "_ldw_patched", False):
        return
    orig = bu.run_command

    def patched(argv, **kw):
        argv = ["--enable-ldw-opt=true" if a == "--enable-ldw-opt=false" else a
                for a in argv]
        return orig(argv, **kw)

    patched._ldw_patched = True
    bu.run_command = patched


def _build_program(plan, mask_counts):
    import concourse.mybir as mybir
    import concourse.tile as tile
    from concourse import bacc

    F32 = mybir.dt.float32
    F32R = mybir.dt.float32r
    MM = mybir.dt.bfloat16
    AF = mybir.ActivationFunctionType
    OP = mybir.AluOpType

    n_mask = sum(mask_counts)

    _enable_ldw_opt()
    nc = bacc.Bacc("TRN2", target_bir_lowering=False, debug=False,
                   num_devices=NCORES)
    hT = nc.dram_tensor("hT", [H, S], MM, kind="ExternalInput").ap()
    wqT = nc.dram_tensor("wqT", [H, QH * HD], MM, kind="ExternalInput").ap()
    wkT = nc.dram_tensor("wkT", [H, HD], MM, kind="ExternalInput").ap()
    wvT = nc.dram_tensor("wvT", [H, HD], MM, kind="ExternalInput").ap()
    woT = nc.dram_tensor("woT", [QH * HD, H], MM, kind="ExternalInput").ap()
    cosT = nc.dram_tensor("cosT", [HD, S], F32, kind="ExternalInput").ap()
    sinT = nc.dram_tensor("sinT", [HD, S], F32, kind="ExternalInput").ap()
    sgn = nc.dram_tensor("sgn", [HD, 1], F32, kind="ExternalInput").ap()
    qc = nc.dram_tensor("qc", [1, QH * SC], F32, kind="ExternalInput").ap()
    iwq = nc.dram_tensor("iwq", [HD, 1], MM, kind="ExternalInput").ap()
    iwk = nc.dram_tensor("iwk", [HD, 1], MM, kind="ExternalInput").ap()
    if n_mask:
        mblk = nc.dram_tensor("mblk", [n_mask, KT, SC], F32,
                              kind="ExternalInput").ap()
        mtri = nc.dram_tensor("mtri", [KT, KT], MM, kind="ExternalInput").ap()
        mhot = nc.dram_tensor("mhot", [n_mask, KT, SC], MM,
                              kind="ExternalInput").ap()
    out = nc.dram_tensor("out", [S, H], F32, kind="ExternalOutput").ap()

    with tile.TileContext(nc) as tc:
        _emit(nc, tc, locals(), plan, mask_counts, MM, F32, F32R, AF, OP)
    nc.compile()
    return nc


def _emit(nc, tc, T, plan, mask_counts, MM, F32, F32R, AF, OP):
    from contextlib import ExitStack

    hT, wqT, wkT, wvT, woT = T["hT"], T["wqT"], T["wkT"], T["wvT"], T["woT"]
    cosT, sinT, sgn, qc = T["cosT"], T["sinT"], T["sgn"], T["qc"]
    iwq, iwk, out = T["iwq"], T["iwk"], T["out"]
    mblk = T.get("mblk")
    mtri = T.get("mtri")
    mhot = T.get("mhot")

    ctx = ExitStack()
    with ctx:
        const = ctx.enter_context(tc.tile_pool(name="const", bufs=1))
        wpool = ctx.enter_context(tc.tile_pool(name="w", bufs=1))
        persist = ctx.enter_context(tc.tile_pool(name="persist", bufs=1))
        hpool = ctx.enter_context(tc.tile_pool(name="h", bufs=18))
        mpool = ctx.enter_context(tc.tile_pool(name="m", bufs=6))
        spool = ctx.enter_context(tc.tile_pool(name="s", bufs=2))
        epool = ctx.enter_context(tc.tile_pool(name="e", bufs=3))
        atpool = ctx.enter_context(tc.tile_pool(name="at", bufs=8))
        opool_sb = ctx.enter_context(tc.tile_pool(name="osb", bufs=3))
        ps_mm = ctx.enter_context(tc.tile_pool(name="psmm", bufs=2, space="PSUM"))
        ps_v = ctx.enter_context(tc.tile_pool(name="psv", bufs=1, space="PSUM"))
        ps_pv = ctx.enter_context(tc.tile_pool(name="pspv", bufs=2, space="PSUM"))
        ps_es = ctx.enter_context(tc.tile_pool(name="pses", bufs=1, space="PSUM"))
        ps_o = ctx.enter_context(tc.tile_pool(name="pso", bufs=2, space="PSUM"))

        # ---- persistent tiles (loads emitted by the driver below) -------
        wq_t = wpool.tile([128, NHC * QH * HD], MM, tag="wq")
        wk_t = wpool.tile([128, NHC * HD], MM, tag="wk")
        wv_t = wpool.tile([128, NHC * HD], MM, tag="wv")
        wo_t = wpool.tile([128, QH * H], MM, tag="wo")
        cos_t = wpool.tile([128, S], F32, tag="cos")
        sin_t = wpool.tile([128, S], F32, tag="sin")
        sgn_t = const.tile([128, 1], F32, tag="sgn")
        qc_t = const.tile([1, QH * SC], F32, tag="qc")
        iwq_t = const.tile([128, 1], MM, tag="iwq")
        iwk_t = const.tile([128, 1], MM, tag="iwk")
        ones_t = const.tile([128, 1], MM, tag="ones")
        eps_t = const.tile([1, 1], F32, tag="eps")
        tri_t = (const.tile([128, KT], MM, tag="tri", name="tri")
                 if mtri is not None else None)
        khat = persist.tile([128, S], MM, tag="khat")
        v_sb = persist.tile([128, S], MM, tag="v")
        qhat = [persist.tile([128, S], MM, name=f"qhat{i}", tag=f"qhat{i}")
                for i in range(QH)]

        def load_early():
            for c in range(NHC):
                nc.sync.dma_start(wq_t[:, c * QH * HD:(c + 1) * QH * HD],
                                  wqT[c * HC:(c + 1) * HC, :])
                nc.sync.dma_start(wk_t[:, c * HD:(c + 1) * HD],
                                  wkT[c * HC:(c + 1) * HC, :])
                nc.sync.dma_start(wv_t[:, c * HD:(c + 1) * HD],
                                  wvT[c * HC:(c + 1) * HC, :])
            nc.sync.dma_start(sgn_t[:], sgn[:])
            nc.sync.dma_start(qc_t[:], qc[:])
            nc.sync.dma_start(iwq_t[:], iwq[:])
            nc.sync.dma_start(iwk_t[:], iwk[:])
            nc.vector.memset(ones_t[:], 1.0)
            nc.vector.memset(eps_t[:], EPS)
            if mtri is not None:
                nc.sync.dma_start(tri_t[:], mtri[:])
            nc.sync.dma_start(cos_t[:], cosT[:])
            nc.sync.dma_start(sin_t[:], sinT[:])

        def load_wo():
            for f in range(QH):
                nc.sync.dma_start(wo_t[:, f * H:(f + 1) * H],
                                  woT[f * HD:(f + 1) * HD, :])

        # per-(head,chunk) norm+rope: PSUM proj tile -> SBUF hat tile
        def norm_rope(pp, sc, iw_t, qconst_slice, hat_dst):
            sh = spool.tile([128, SC], F32, tag="sh")
            nc.vector.tensor_copy(sh[0:64, :], pp[64:128, :])
            nc.vector.tensor_copy(sh[64:128, :], pp[0:64, :])
            # var from the rotated copy (rotation-invariant; iw_t rows are
            # pre-rotated host-side to match)
            sq = spool.tile([128, SC], MM, tag="sq")
            nc.vector.tensor_mul(sq[:], sh[:], sh[:])
            var = ps_mm.tile([1, SC], F32, tag="mm")
            nc.tensor.matmul(var[:], iw_t[:], sq[:], start=True, stop=True)
            sd = spool.tile([1, SC], F32, tag="sd")
            nc.scalar.activation(sd[:], var[:], AF.Sqrt, bias=eps_t[:])
            rs = spool.tile([1, SC], F32, tag="rs")
            nc.vector.reciprocal_approx_fast(rs[:], sd[:])
            if qconst_slice is not None:
                nc.vector.tensor_mul(rs[:], rs[:], qconst_slice)
            bb = spool.tile([128, SC], F32, tag="bb")
            nc.gpsimd.partition_broadcast(bb[:], rs[:], 128)
            tt = spool.tile([128, SC], F32, tag="tt")
            nc.vector.scalar_tensor_tensor(
                tt[:], sh[:], sgn_t[:], sin_t[:, sc * SC:(sc + 1) * SC],
                op0=OP.mult, op1=OP.mult)
            uu = spool.tile([128, SC], F32, tag="uu")
            nc.vector.tensor_mul(uu[:], pp[:], cos_t[:, sc * SC:(sc + 1) * SC])
            nc.vector.tensor_add(tt[:], tt[:], uu[:])
            nc.vector.tensor_mul(hat_dst, tt[:], bb[:])

        # ---- projections, per s-chunk -----------------------------------
        def hts_load(sc):
            hts = []
            for c in range(NHC):
                t = hpool.tile([128, SC], MM, tag="ht", name="ht")
                nc.sync.dma_start(t[:], hT[c * HC:(c + 1) * HC,
                                           sc * SC:(sc + 1) * SC])
                hts.append(t)
            return hts

        def proj_chunk(sc, hts):
            # k-proj first: its first matmul only needs hts[0]
            kp = ps_mm.tile([128, SC], F32, tag="mm")
            for c in range(NHC):
                nc.tensor.matmul(kp[:], wk_t[:, c * HD:(c + 1) * HD],
                                 hts[c][:], start=(c == 0), stop=(c == NHC - 1))
            norm_rope(kp, sc, iwk_t, None, khat[:, sc * SC:(sc + 1) * SC])
            # q-proj per head
            for hd in range(QH):
                qp = ps_mm.tile([128, SC], F32, tag="mm")
                for c in range(NHC):
                    base = c * QH * HD + hd * HD
                    nc.tensor.matmul(qp[:], wq_t[:, base:base + HD], hts[c][:],
                                     start=(c == 0), stop=(c == NHC - 1))
                norm_rope(qp, sc, iwq_t,
                          qc_t[:, hd * SC:(hd + 1) * SC],
                          qhat[hd][:, sc * SC:(sc + 1) * SC])
            # v-proj: natural [s, d] layout, N=128 matmuls
            for ss in range(4):
                vp = ps_v.tile([128, HD], F32, tag="v")
                for c in range(NHC):
                    nc.tensor.matmul(vp[:], hts[c][:, ss * 128:(ss + 1) * 128],
                                     wv_t[:, c * HD:(c + 1) * HD],
                                     start=(c == 0), stop=(c == NHC - 1))
                col = (sc * 4 + ss) * 128
                nc.vector.tensor_copy(v_sb[:, col:col + 128], vp[:])

        # ---- attention + o_proj, per q-tile ------------------------------
        mask_starts = [sum(mask_counts[:i]) for i in range(NSC)]

        def attn_qtile(qi):
            mask_idx = mask_starts[qi]
            kts = [kt for kt in range(NKT) if plan[qi][kt] != "skip"]
            # load this q-tile's mask blocks (shared across heads)
            mtiles = {}
            for kt in kts:
                if plan[qi][kt] == "step":
                    mt = mpool.tile([128, SC], MM, tag="maskh", name="mh")
                    nc.sync.dma_start(mt[:], mhot[mask_idx])
                    mtiles[kt] = ("step", mt)
                    mask_idx += 1
                elif plan[qi][kt] == "mask":
                    mt = mpool.tile([128, SC], F32, tag="mask", name="mk")
                    nc.sync.dma_start(mt[:], mblk[mask_idx])
                    mtiles[kt] = ("mask", mt)
                    mask_idx += 1
            ats = []
            for hd in range(QH):
                qsl = qhat[hd][:, qi * SC:(qi + 1) * SC]
                pv = ps_pv.tile([128, SC], F32, tag="pv")
                es = ps_es.tile([1, SC], F32, tag="es")
                sts = {}
                # pipeline QK^T one k-tile ahead of exp/PV
                for j, kt in enumerate(kts):
                    st = ps_mm.tile([128, SC], F32, tag="mm")
                    kind, mt = mtiles.get(kt, (None, None))
                    nc.tensor.matmul(st[:], khat[:, kt * 128:(kt + 1) * 128],
                                     qsl, start=True, stop=(kind != "step"))
                    if kind == "step":
                        nc.tensor.matmul(st[:], tri_t[:], mt[:],
                                         start=False, stop=True)
                    elif kind == "mask":
                        nc.vector.tensor_add(st[:], st[:], mt[:])
                    sts[j] = st
                    if j >= 1:
                        _attn_tail(nc, j - 1, kts, sts, es, pv, v_sb,
                                   ones_t, MM, AF, epool)
                _attn_tail(nc, len(kts) - 1, kts, sts, es, pv, v_sb,
                           ones_t, MM, AF, epool)
                rs = spool.tile([1, SC], F32, tag="ars")
                nc.vector.reciprocal_approx_fast(rs[:], es[:])
                bb = spool.tile([128, SC], F32, tag="abb")
                nc.gpsimd.partition_broadcast(bb[:], rs[:], 128)
                at = atpool.tile([128, SC], MM, tag="at")
                nc.vector.tensor_mul(at[:], pv[:], bb[:])
                ats.append(at)
            # o_proj for this q-tile
            for ss in range(4):
                for ho in range(4):
                    op_t = ps_o.tile([128, SC], F32, tag="o")
                    for hd in range(QH):
                        nc.tensor.matmul(
                            op_t[:],
                            ats[hd][:, ss * 128:(ss + 1) * 128],
                            wo_t[:, hd * H + ho * SC:hd * H + (ho + 1) * SC],
                            start=(hd == 0), stop=(hd == QH - 1))
                    ob = opool_sb.tile([128, SC], F32, tag="osb")
                    nc.vector.tensor_copy(ob[:], op_t[:])
                    nc.sync.dma_start(
                        out[qi * SC + ss * 128:qi * SC + (ss + 1) * 128,
                            ho * SC:(ho + 1) * SC],
                        ob[:])

        # ---- driver: software-pipelined phase order ----------------------
        hts0 = hts_load(0)
        load_early()
        proj_chunk(0, hts0)
        hts1 = hts_load(1)
        proj_chunk(1, hts1)
        load_wo()
        attn_qtile(0)
        hts2 = hts_load(2)
        proj_chunk(2, hts2)
        attn_qtile(1)
        hts3 = hts_load(3)
        proj_chunk(3, hts3)
        attn_qtile(2)
        attn_qtile(3)


def _attn_tail(nc, j, kts, sts, es, pv, v_sb, ones_t, MM, AF, epool):
    """exp + PV + PSUM-accumulated denominator for pipelined k-tile j."""
    kt = kts[j]
    st = sts.pop(j)
    ex = epool.tile([128, SC], MM, tag="ex", name="ex")
    nc.scalar.activation(ex[:], st[:], AF.Exp)
    last = j == len(kts) - 1
    nc.tensor.matmul(pv[:], v_sb[:, kt * 128:(kt + 1) * 128], ex[:],
                     start=(j == 0), stop=last)
    nc.tensor.matmul(es[:], ones_t[:], ex[:], start=(j == 0), stop=last)


def _is_step(blk):
    """True if every batch/column is 0 for k < f and exactly -1e9 for k >= f."""
    isneg = blk == np.float32(-1e9)
    iszero = blk == 0.0
    if not (isneg | iszero).all():
        return False
    # per (b, q): suffix property along k
    f = isneg.argmax(axis=-1) + 0  # first masked k (0 if none masked)
    any_neg = isneg.any(axis=-1)
    kk = np.arange(blk.shape[-1])
    want = np.where(any_neg[..., None], kk[None, None] >= f[..., None], False)
    return bool((isneg == want).all())


def _mask_plan(mask):
    """Classify [qi][kt] blocks of the (q,k) mask, unified across batch."""
    plan = []
    for qi in range(NSC):
        row = []
        for kt in range(NKT):
            blk = mask[:, 0, qi * SC:(qi + 1) * SC, kt * KT:(kt + 1) * KT]
            if (blk <= SKIP_THRESH).all():
                row.append("skip")
            elif (blk == 0.0).all():
                row.append("zero")
            elif _is_step(blk):
                row.append("step")
            else:
                row.append("mask")
        # guard: a q-tile with no included block would divide by zero
        if all(s == "skip" for s in row):
            row[0] = "mask"
        plan.append(row)
    return plan


def kernel(hidden_states, cos, sin, attention_mask, wq, wk, wv, wo,
           q_norm_w, k_norm_w, ssmax_scale):
    global LAST_EXEC_NS
    import os
    import ml_dtypes
    from concourse.bass_utils import run_bass_kernel_spmd

    f32 = np.float32
    hidden_states = np.asarray(hidden_states, f32)
    cos = np.asarray(cos, f32)
    sin = np.asarray(sin, f32)
    attention_mask = np.asarray(attention_mask, f32)
    wq = np.asarray(wq, f32)
    wk = np.asarray(wk, f32)
    wv = np.asarray(wv, f32)
    wo = np.asarray(wo, f32)
    q_norm_w = np.asarray(q_norm_w, f32)
    k_norm_w = np.asarray(k_norm_w, f32)
    ssmax = np.asarray(ssmax_scale, f32).reshape(NH)

    plan = _mask_plan(attention_mask)
    mask_counts = [sum(1 for s in row if s in ("mask", "step")) for row in plan]
    key = (tuple(tuple(r) for r in plan),)
    if key not in _compiled_cache:
        _compiled_cache[key] = _build_program(plan, mask_counts)
    nc = _compiled_cache[key]

    bf16 = ml_dtypes.bfloat16
    qw = np.tile(q_norm_w, QH)
    sgn_np = np.concatenate([-np.ones(64, f32), np.ones(64, f32)])[:, None]
    iwq_np = np.roll(1.0 / (HD * q_norm_w ** 2), -64).astype(bf16)[:, None]
    iwk_np = np.roll(1.0 / (HD * k_norm_w ** 2), -64).astype(bf16)[:, None]
    cosT_np = np.ascontiguousarray(cos.T)
    sinT_np = np.ascontiguousarray(sin.T)

    in_maps = []
    for core in range(NCORES):
        b, g = divmod(core, TP)
        hTm = np.ascontiguousarray(hidden_states[b].T).astype(bf16)
        wq_s = wq[g * QH * HD:(g + 1) * QH * HD] * qw[:, None]
        wk_s = wk[g * HD:(g + 1) * HD] * k_norm_w[:, None]
        wv_s = wv[g * HD:(g + 1) * HD]
        wo_s = wo[:, g * QH * HD:(g + 1) * QH * HD]
        qc_np = np.empty((1, QH * SC), f32)
        for i in range(QH):
            qc_np[0, i * SC:(i + 1) * SC] = (
                ssmax[g * QH + i] * math.log(S) / math.sqrt(HD))
        m = {
            "hT": hTm,
            "wqT": np.ascontiguousarray(wq_s.T).astype(bf16),
            "wkT": np.ascontiguousarray(wk_s.T).astype(bf16),
            "wvT": np.ascontiguousarray(wv_s.T).astype(bf16),
            "woT": np.ascontiguousarray(wo_s.T).astype(bf16),
            "cosT": cosT_np, "sinT": sinT_np, "sgn": sgn_np,
            "qc": qc_np, "iwq": iwq_np, "iwk": iwk_np,
        }
        n_mask = sum(mask_counts)
        if n_mask:
            blocks = np.zeros((n_mask, KT, SC), f32)
            hots = np.zeros((n_mask, KT, SC), f32)
            i = 0
            for qi in range(NSC):
                for kt in range(NKT):
                    kind = plan[qi][kt]
                    if kind not in ("mask", "step"):
                        continue
                    blkT = attention_mask[
                        b, 0, qi * SC:(qi + 1) * SC,
                        kt * KT:(kt + 1) * KT].T
                    if kind == "mask":
                        blocks[i] = blkT
                    else:
                        isneg = blkT == np.float32(-1e9)
                        f = isneg.argmax(axis=0)
                        anyneg = isneg.any(axis=0)
                        qsel = np.nonzero(anyneg)[0]
                        hots[i][f[qsel], qsel] = 1.0
                    i += 1
            m["mblk"] = blocks
            m["mhot"] = hots.astype(bf16)
            # tri[r, k] = -1e9 * [k >= r]; lhsT layout [r(part), k(free)]
            tri = (-1e9 * (np.arange(KT)[None, :] >= np.arange(KT)[:, None]))
            m["mtri"] = np.ascontiguousarray(tri).astype(bf16)
        in_maps.append(m)

    trace = bool(int(os.environ.get("BASS_KERNEL_TRACE", "0")))
    res = run_bass_kernel_spmd(nc, in_maps, list(range(NCORES)), trace=trace)
    LAST_EXEC_NS = res.exec_time_ns
    globals()["LAST_RESULTS"] = res

    final = np.zeros((B, S, H), f32)
    for core in range(NCORES):
        b = core // TP
        final[b] += res.results[core]["out"]
    return final
